# revision 26
# baseline (speedup 1.0000x reference)
"""AudioVisualAlignmentLoss kernel for Trainium2 (8 NeuronCores, SPMD).

Strategy: pure data parallelism — batch B=32 is split 4-per-core across 8
cores. Host-side prep is limited to index/layout work:
  * segment bounds replicated bit-exactly from the reference's
    jnp.linspace(...).astype(int32) (round-to-nearest-even on this backend),
  * per-frame segment id with mask/length folded in (-1 = inactive),
  * audio split into a bf16 hi/lo pair (hi+lo == fp32 to ~2^-18 rel) laid
    out as [T, 257] = [hi | lo | 1] rows so one matmul per 128-frame chunk
    produces segment sums (hi+lo) and counts (ones column) in PSUM.

Device per chunk: DVE is_equal builds the one-hot membership [128t, 64s]
in bf16; PE accumulates onehot.T @ [hi|lo|1] into PSUM [64s, 257].
Per batch tail: scale by 1/count, PE transpose, project with W^T, LayerNorm
(bn_stats), L2-normalize, dot with visual embedding, weight -> 64 partials.
Host sums partials and divides by the weight sum.

All small tensors ride in one packed "meta" [128, META_COLS] f32 DMA so
instructions depend on a single DMA semaphore (walrus TensorScalarPtr
codegen allows only one sync-wait slot).
"""

import numpy as np
import ml_dtypes

import concourse.bass as bass
import concourse.mybir as mybir
import concourse.tile as tile
from concourse.bass_utils import run_bass_kernel_spmd

BF16 = ml_dtypes.bfloat16

B, T, D, S, E = 32, 8192, 128, 64, 128
NCORES = 8
BL = B // NCORES          # batches per core
NCH = T // 128            # 128-frame chunks per batch
G = 16                    # chunks per DMA group
AUGW = D + 1              # hi | ones (bf16 plane)
LOW = D                    # lo plane (fp8 e4m3, prescaled by 256)
LO_SCALE = 1.0 / 256.0
LN_EPS = 1e-5
NORM_EPS = 1e-12

# meta column layout (f32, [128, META_COLS])
MC_WT = 0                  # [128, 128] W^T (wt[d, e] = W[e, d])
MC_SEG = MC_WT + E         # [128, BL*NCH] seg ids, col = b*NCH + c
MC_SIOTA = MC_SEG + BL * NCH   # [128, S] column index
MC_BVEC = MC_SIOTA + S     # [64, E] b_lin broadcast (partitions 0:S)
MC_GVEC = MC_BVEC + E      # [64, E] gamma broadcast
MC_BETA = MC_GVEC + E      # [64, E] beta broadcast
MC_IDENT = MC_BETA + E     # [64, 64] identity
MC_WGT = MC_IDENT + S      # [64, BL] sync*seg_mask, col b
MC_VIS = MC_WGT + BL       # [64, BL*E] visual embed, col = b*E + e
META_COLS = MC_VIS + BL * E

_NC_CACHE = None


def build_nc(legalize=True):
    f32 = mybir.dt.float32
    bf16 = mybir.dt.bfloat16
    AL = mybir.AluOpType

    nc = bass.Bass()
    aug = nc.declare_dram_parameter("aug", [BL, 128, NCH, AUGW], bf16, isOutput=False)
    auglo = nc.declare_dram_parameter("auglo", [BL, 128, NCH, LOW], mybir.dt.float8e4, isOutput=False)
    meta = nc.declare_dram_parameter("meta", [128, META_COLS], f32, isOutput=False)
    out = nc.declare_dram_parameter("out", [S, BL], f32, isOutput=True)

    with tile.TileContext(nc) as tc:
        with (
            tc.tile_pool(name="consts", bufs=1) as consts,
            tc.tile_pool(name="aug_p", bufs=6) as aug_p,
            tc.tile_pool(name="oh_p", bufs=4) as oh_p,
            tc.tile_pool(name="tail", bufs=2) as tail,
            tc.tile_pool(name="ps_seg", bufs=2, space="PSUM") as ps_seg_p,
            tc.tile_pool(name="ps_lo", bufs=2, space="PSUM") as ps_lo_p,
            tc.tile_pool(name="ps_t", bufs=2, space="PSUM") as ps_t_p,
            tc.tile_pool(name="ps_x", bufs=2, space="PSUM") as ps_x_p,
        ):
            meta_sb = consts.tile([128, META_COLS], f32)
            nc.sync.dma_start(out=meta_sb, in_=meta[:])
            wt_v = meta_sb[:, MC_WT:MC_WT + E]
            siota_v = meta_sb[:, MC_SIOTA:MC_SIOTA + S]
            bvec_v = meta_sb[0:S, MC_BVEC:MC_BVEC + E]
            gvec_v = meta_sb[0:S, MC_GVEC:MC_GVEC + E]
            beta_v = meta_sb[0:S, MC_BETA:MC_BETA + E]
            ident_v = meta_sb[0:S, MC_IDENT:MC_IDENT + S]

            eps_sb = consts.tile([S, 1], f32)
            nc.vector.memset(eps_sb, LN_EPS)
            out_sb = consts.tile([S, BL], f32)

            # one-hot membership, all batches up front, in 16-chunk pieces:
            # oh_all[p, c, s] = (segid[p, c] == s)
            oh_tiles = []
            for b in range(BL):
                oh_all = oh_p.tile([128, NCH, S], bf16)
                oh_tiles.append(oh_all)
                pieces = [4, 12, 16, 16, 16] if b == 0 else [16, 16, 16, 16]
                c0 = 0
                for H in pieces:
                    c1 = c0 + H
                    seg_h = meta_sb[:, MC_SEG + b * NCH + c0:MC_SEG + b * NCH + c1]
                    nc.vector.tensor_tensor(
                        out=oh_all[:, c0:c1, :],
                        in0=seg_h.rearrange("p (c o) -> p c o", o=1).broadcast_to([128, H, S]),
                        in1=siota_v.rearrange("p (o s) -> p o s", o=1).broadcast_to([128, H, S]),
                        op=AL.is_equal,
                    )
                    c0 = c1

            def tail_chain(b, ps_pair):
                ps, pslo = ps_pair
                # segment means: (hi-part + lo-part/256) / max(count, 1)
                # (PSUM has one DVE read port: bounce hi to SBUF via ACT first)
                seg_hi = tail.tile([S, D], f32, tag="seg_hi")
                nc.scalar.copy(out=seg_hi, in_=ps[:, 0:D])
                lo_s = tail.tile([S, D], f32, tag="lo_s")
                nc.vector.tensor_scalar_mul(out=lo_s, in0=pslo[:, 0:D], scalar1=LO_SCALE)
                seg_sum = tail.tile([S, D], f32, tag="seg_sum")
                nc.vector.tensor_add(out=seg_sum, in0=seg_hi, in1=lo_s)
                inv = tail.tile([S, 1], f32, tag="inv")
                nc.vector.tensor_scalar_max(out=inv, in0=ps[:, D:D + 1], scalar1=1.0)
                nc.vector.reciprocal(out=inv, in_=inv)
                aseg = tail.tile([S, D], f32, tag="aseg")
                nc.vector.tensor_scalar_mul(out=aseg, in0=seg_sum, scalar1=inv)

                # x = aseg @ W.T + b  (via PE transpose then matmul with W^T)
                ps_t = ps_t_p.tile([D, S], f32)
                nc.tensor.transpose(ps_t, aseg, ident_v)
                asegT = tail.tile([D, S], f32, tag="asegT")
                nc.scalar.copy(out=asegT, in_=ps_t)
                ps_x = ps_x_p.tile([S, E], f32)
                nc.tensor.matmul(ps_x, lhsT=asegT, rhs=wt_v, start=True, stop=True)
                x = tail.tile([S, E], f32, tag="x")
                nc.vector.tensor_add(out=x, in0=ps_x, in1=bvec_v)

                # LayerNorm
                stats = tail.tile([S, 6], f32, tag="stats")
                nc.vector.bn_stats(out=stats, in_=x)
                mv = tail.tile([S, 2], f32, tag="mv")
                nc.vector.bn_aggr(out=mv, in_=stats)
                rstd = tail.tile([S, 1], f32, tag="rstd")
                nc.scalar.activation(
                    out=rstd, in_=mv[:, 1:2],
                    func=mybir.ActivationFunctionType.Sqrt, bias=eps_sb, scale=1.0,
                )
                nc.vector.reciprocal(out=rstd, in_=rstd)
                y = tail.tile([S, E], f32, tag="y")
                nc.vector.tensor_scalar(
                    out=y, in0=x, scalar1=mv[:, 0:1], scalar2=rstd,
                    op0=AL.subtract, op1=AL.mult,
                )
                nc.vector.tensor_mul(out=y, in0=y, in1=gvec_v)
                nc.vector.tensor_add(out=y, in0=y, in1=beta_v)

                # L2 norm + weighted cosine
                yy = tail.tile([S, E], f32, tag="yy")
                nc.vector.tensor_mul(out=yy, in0=y, in1=y)
                ss = tail.tile([S, 1], f32, tag="ss")
                nc.vector.reduce_sum(out=ss, in_=yy, axis=mybir.AxisListType.X)
                rn = tail.tile([S, 1], f32, tag="rn")
                nc.scalar.activation(out=rn, in_=ss, func=mybir.ActivationFunctionType.Sqrt)
                nc.vector.tensor_scalar_max(out=rn, in0=rn, scalar1=NORM_EPS)
                nc.vector.reciprocal(out=rn, in_=rn)
                pv = tail.tile([S, E], f32, tag="pv")
                vis_v = meta_sb[0:S, MC_VIS + b * E:MC_VIS + (b + 1) * E]
                nc.vector.tensor_mul(out=pv, in0=y, in1=vis_v)
                dot = tail.tile([S, 1], f32, tag="dot")
                nc.vector.reduce_sum(out=dot, in_=pv, axis=mybir.AxisListType.X)
                nc.vector.tensor_mul(out=dot, in0=dot, in1=rn)
                wgt_col = meta_sb[0:S, MC_WGT + b:MC_WGT + b + 1]
                nc.vector.tensor_mul(out=out_sb[:, b:b + 1], in0=dot, in1=wgt_col)

            # stream each batch's accumulation; emit the previous batch's
            # tail AFTER this batch's matmuls so the PE FIFO never blocks on
            # the DVE tail chain (software pipelining, one batch deep)
            ps_tiles = []
            for b in range(BL):
                oh_all = oh_tiles[b]
                ps = ps_seg_p.tile([S, AUGW], f32)
                pslo = ps_lo_p.tile([S, LOW], f32)
                ps_tiles.append((ps, pslo))
                for g in range(NCH // G):
                    aug_sb = aug_p.tile([128, G, AUGW], bf16)
                    nc.sync.dma_start(out=aug_sb, in_=aug[b][:, g * G:(g + 1) * G, :])
                    auglo_sb = aug_p.tile([128, G, LOW], mybir.dt.float8e4, tag="auglo")
                    nc.sync.dma_start(out=auglo_sb, in_=auglo[b][:, g * G:(g + 1) * G, :])
                    for j in range(G):
                        c = g * G + j
                        nc.tensor.matmul(
                            ps, lhsT=oh_all[:, c, :], rhs=aug_sb[:, j, :],
                            start=(c == 0), stop=(c == NCH - 1),
                        )
                        nc.tensor.matmul(
                            pslo, lhsT=oh_all[:, c, :], rhs=auglo_sb[:, j, :],
                            start=(c == 0), stop=(c == NCH - 1),
                        )
                if b >= 1:
                    tail_chain(b - 1, ps_tiles[b - 1])
            tail_chain(BL - 1, ps_tiles[BL - 1])

            nc.sync.dma_start(out=out[:], in_=out_sb)

    if legalize:
        _legalize_multiwait(nc)
    return nc


def _legalize_multiwait(nc):
    """This container's walrus codegen encodes at most ONE sync-wait per TPB
    instruction. Tile emits instructions with several waits; split the
    excess onto standalone wait-only EventSemaphore instructions inserted
    immediately before, on the same engine (sequencers execute per-engine
    in block order, so semantics are identical)."""
    wid = 0
    for fn in nc.m.functions:
        for blk in fn.blocks:
            out_list = []
            for inst in blk.instructions:
                si = inst.sync_info
                if si is not None and len(si.on_wait) > 1:
                    for w in si.on_wait[:-1]:
                        wid += 1
                        nop = mybir.InstEventSemaphore(
                            name=f"W-split-{wid}",
                            engine=inst.engine,
                            ins=[], outs=[],
                            sync_info=mybir.SyncInfo(on_wait=[w], on_update=[]),
                        )
                        out_list.append(nop)
                    inst.sync_info = mybir.SyncInfo(
                        on_wait=[si.on_wait[-1]], on_update=si.on_update)
                out_list.append(inst)
            blk.instructions[:] = out_list


def get_nc():
    global _NC_CACHE
    if _NC_CACHE is None:
        _NC_CACHE = build_nc()
    return _NC_CACHE


def segment_bounds(length):
    """Replicates jnp.linspace(0, L, S+1).astype(int32) as executed by the
    reference on this backend: f32 lerp, then round-half-even f32->s32."""
    lf = np.asarray(length).astype(np.float32)
    i = np.arange(S + 1, dtype=np.float32)
    bf = (i[None, :] * (lf / np.float32(S))[:, None]).astype(np.float32)
    return np.rint(bf).astype(np.int32)


def make_inputs(audio, visual_embed, sync_score, mask, length, seg_mask,
                W, b_lin, gamma, beta):
    audio = np.asarray(audio, dtype=np.float32)
    mask = np.asarray(mask, dtype=np.int32)
    length = np.asarray(length, dtype=np.int32)

    bounds = segment_bounds(length)            # [B, S+1]
    ends = bounds[:, 1:]                       # [B, S]
    t = np.arange(T, dtype=np.int32)
    seg_id = np.empty((B, T), np.int32)
    for bb in range(B):
        s = np.searchsorted(ends[bb], t, side="right").astype(np.int32)
        s[s >= S] = -1
        s[t >= length[bb]] = -1
        s[mask[bb] == 0] = -1
        seg_id[bb] = s
    # [B, 128(p), NCH(c)] with t = c*128 + p
    segid_t = np.ascontiguousarray(
        seg_id.reshape(B, NCH, 128).transpose(0, 2, 1)).astype(np.float32)

    hi = audio.astype(BF16)
    lo = ((audio - hi.astype(np.float32)) * 256.0).astype(ml_dtypes.float8_e4m3)
    # transposed layout: aug[b, p, c, :] = [hi | 1] of frame t = c*128 + p
    # (big contiguous DMA descriptors); lo plane separate in fp8 e4m3 x256
    aug = np.empty((B, 128, NCH, AUGW), dtype=BF16)
    aug[:, :, :, 0:D] = hi.reshape(B, NCH, 128, D).transpose(0, 2, 1, 3)
    aug[:, :, :, D] = BF16(1.0)
    auglo = np.ascontiguousarray(lo.reshape(B, NCH, 128, D).transpose(0, 2, 1, 3))

    visf = np.asarray(visual_embed, dtype=np.float32)
    wgt = np.asarray(sync_score, np.float32) * np.asarray(seg_mask, np.int32).astype(np.float32)

    meta_common = np.zeros((128, META_COLS), np.float32)
    meta_common[:, MC_WT:MC_WT + E] = np.asarray(W, np.float32).T
    meta_common[:, MC_SIOTA:MC_SIOTA + S] = np.arange(S, dtype=np.float32)[None, :]
    meta_common[0:S, MC_BVEC:MC_BVEC + E] = np.asarray(b_lin, np.float32)[None, :]
    meta_common[0:S, MC_GVEC:MC_GVEC + E] = np.asarray(gamma, np.float32)[None, :]
    meta_common[0:S, MC_BETA:MC_BETA + E] = np.asarray(beta, np.float32)[None, :]
    meta_common[0:S, MC_IDENT:MC_IDENT + S] = np.eye(S, dtype=np.float32)

    in_maps = []
    for k in range(NCORES):
        sl = slice(k * BL, (k + 1) * BL)
        meta = meta_common.copy()
        meta[:, MC_SEG:MC_SEG + BL * NCH] = (
            segid_t[sl].transpose(1, 0, 2).reshape(128, BL * NCH))
        meta[0:S, MC_WGT:MC_WGT + BL] = wgt[sl].T
        meta[0:S, MC_VIS:MC_VIS + BL * E] = (
            visf[sl].transpose(1, 0, 2).reshape(S, BL * E))
        in_maps.append({
            "aug": np.ascontiguousarray(aug[sl]),
            "auglo": np.ascontiguousarray(auglo[sl]),
            "meta": meta,
        })
    return in_maps, wgt


def finish(results, wgt):
    total = 0.0
    for r in results:
        total += float(np.asarray(r["out"], dtype=np.float64).sum())
    wsum = max(float(wgt.astype(np.float64).sum()), 1e-6)
    return np.float32(-(total / wsum))


def run(trace=False, **inputs):
    in_maps, wgt = make_inputs(**inputs)
    res = run_bass_kernel_spmd(get_nc(), in_maps, list(range(NCORES)), trace=trace)
    loss = finish(res.results, wgt)
    return np.asarray(loss, dtype=np.float32), res


def kernel(**inputs):
    loss, _ = run(trace=False, **inputs)
    return loss


# revision 27
# speedup vs baseline: 1.0088x; 1.0088x over previous
"""AudioVisualAlignmentLoss kernel for Trainium2 (8 NeuronCores, SPMD).

Strategy: pure data parallelism — batch B=32 is split 4-per-core across 8
cores. Host-side prep is limited to index/layout work:
  * segment bounds replicated bit-exactly from the reference's
    jnp.linspace(...).astype(int32) (round-to-nearest-even on this backend),
  * per-frame segment id with mask/length folded in (-1 = inactive),
  * audio split into a bf16 `hi` plane laid out as [hi | 1] rows plus an
    fp8-e4m3 `lo` residual plane prescaled by 256 (hi + lo/256 ~ fp32 to
    ~2^-12 per element; end-to-end loss error ~3e-6), both in a
    partition-major layout so every DMA descriptor reads 2-4KB contiguous.

Device per chunk of 128 frames: one bf16 one-hot membership tile
[128t, 64s] (built per batch by a single broadcast is_equal) is the shared
matmul stationary; two matmuls accumulate onehot.T @ [hi|1] -> PSUM
[64s, 129] and onehot.T @ lo -> PSUM [64s, 128] (segment sums + counts).
Per batch tail (software-pipelined one batch behind the stream): segment
means, PE transpose, projection with W^T, LayerNorm (bn_stats),
L2-normalize, dot with the visual embedding, sync/seg-mask weighting ->
64 partials. Host sums the 8x[64, 4] partials and divides by the weight sum.

All small tensors ride in one packed "meta" [128, META_COLS] f32 DMA that
is first on the HWDGE ring, so dependent instructions need one DMA
semaphore. _legalize_multiwait splits Tile-emitted multi-wait sync_infos
into standalone EventSemaphore waits (this walrus encodes at most one
sync-wait per TPB instruction).
"""

import numpy as np
import ml_dtypes

import concourse.bass as bass
import concourse.mybir as mybir
import concourse.tile as tile
from concourse.bass_utils import run_bass_kernel_spmd

BF16 = ml_dtypes.bfloat16

B, T, D, S, E = 32, 8192, 128, 64, 128
NCORES = 8
BL = B // NCORES          # batches per core
NCH = T // 128            # 128-frame chunks per batch
G = 16                    # chunks per DMA group
AUGW = D + 1              # hi | ones (bf16 plane)
LOW = D                    # lo plane (fp8 e4m3, prescaled by 256)
LO_SCALE = 1.0 / 256.0
LN_EPS = 1e-5
NORM_EPS = 1e-12

# meta column layout (f32, [128, META_COLS])
MC_WT = 0                  # [128, 128] W^T (wt[d, e] = W[e, d])
MC_SEG = MC_WT + E         # [128, BL*NCH] seg ids, col = b*NCH + c
MC_SIOTA = MC_SEG + BL * NCH   # [128, S] column index
MC_BVEC = MC_SIOTA + S     # [64, E] b_lin broadcast (partitions 0:S)
MC_GVEC = MC_BVEC + E      # [64, E] gamma broadcast
MC_BETA = MC_GVEC + E      # [64, E] beta broadcast
MC_IDENT = MC_BETA + E     # [64, 64] identity
MC_WGT = MC_IDENT + S      # [64, BL] sync*seg_mask, col b
MC_VIS = MC_WGT + BL       # [64, BL*E] visual embed, col = b*E + e
META_COLS = MC_VIS + BL * E

_NC_CACHE = None


def build_nc(legalize=True):
    f32 = mybir.dt.float32
    bf16 = mybir.dt.bfloat16
    AL = mybir.AluOpType

    nc = bass.Bass()
    aug = nc.declare_dram_parameter("aug", [BL, 128, NCH, AUGW], bf16, isOutput=False)
    auglo = nc.declare_dram_parameter("auglo", [BL, 128, NCH, LOW], mybir.dt.float8e4, isOutput=False)
    meta = nc.declare_dram_parameter("meta", [128, META_COLS], f32, isOutput=False)
    out = nc.declare_dram_parameter("out", [S, BL], f32, isOutput=True)

    with tile.TileContext(nc) as tc:
        with (
            tc.tile_pool(name="consts", bufs=1) as consts,
            tc.tile_pool(name="aug_p", bufs=6) as aug_p,
            tc.tile_pool(name="oh_p", bufs=4) as oh_p,
            tc.tile_pool(name="tail", bufs=2) as tail,
            tc.tile_pool(name="ps_seg", bufs=2, space="PSUM") as ps_seg_p,
            tc.tile_pool(name="ps_lo", bufs=2, space="PSUM") as ps_lo_p,
            tc.tile_pool(name="ps_t", bufs=2, space="PSUM") as ps_t_p,
            tc.tile_pool(name="ps_x", bufs=2, space="PSUM") as ps_x_p,
        ):
            meta_sb = consts.tile([128, META_COLS], f32)
            nc.sync.dma_start(out=meta_sb, in_=meta[:])
            wt_v = meta_sb[:, MC_WT:MC_WT + E]
            siota_v = meta_sb[:, MC_SIOTA:MC_SIOTA + S]
            bvec_v = meta_sb[0:S, MC_BVEC:MC_BVEC + E]
            gvec_v = meta_sb[0:S, MC_GVEC:MC_GVEC + E]
            beta_v = meta_sb[0:S, MC_BETA:MC_BETA + E]
            ident_v = meta_sb[0:S, MC_IDENT:MC_IDENT + S]

            eps_sb = consts.tile([S, 1], f32)
            nc.vector.memset(eps_sb, LN_EPS)
            out_sb = consts.tile([S, BL], f32)

            # one-hot membership, all batches up front, in 16-chunk pieces:
            # oh_all[p, c, s] = (segid[p, c] == s)
            oh_tiles = []
            for b in range(BL):
                oh_all = oh_p.tile([128, NCH, S], bf16)
                oh_tiles.append(oh_all)
                pieces = [4, 12, 16, 16, 16] if b == 0 else [16, 16, 16, 16]
                c0 = 0
                for H in pieces:
                    c1 = c0 + H
                    seg_h = meta_sb[:, MC_SEG + b * NCH + c0:MC_SEG + b * NCH + c1]
                    nc.vector.tensor_tensor(
                        out=oh_all[:, c0:c1, :],
                        in0=seg_h.rearrange("p (c o) -> p c o", o=1).broadcast_to([128, H, S]),
                        in1=siota_v.rearrange("p (o s) -> p o s", o=1).broadcast_to([128, H, S]),
                        op=AL.is_equal,
                    )
                    c0 = c1

            def tail_chain(b, ps_pair):
                ps, pslo = ps_pair
                # segment means: (hi-part + lo-part/256) / max(count, 1)
                # (PSUM has one DVE read port: bounce hi to SBUF via ACT first)
                seg_hi = tail.tile([S, D], f32, tag="seg_hi")
                nc.scalar.copy(out=seg_hi, in_=ps[:, 0:D])
                lo_s = tail.tile([S, D], f32, tag="lo_s")
                nc.vector.tensor_scalar_mul(out=lo_s, in0=pslo[:, 0:D], scalar1=LO_SCALE)
                seg_sum = tail.tile([S, D], f32, tag="seg_sum")
                nc.vector.tensor_add(out=seg_sum, in0=seg_hi, in1=lo_s)
                inv = tail.tile([S, 1], f32, tag="inv")
                nc.vector.tensor_scalar_max(out=inv, in0=ps[:, D:D + 1], scalar1=1.0)
                nc.vector.reciprocal(out=inv, in_=inv)
                aseg = tail.tile([S, D], f32, tag="aseg")
                nc.vector.tensor_scalar_mul(out=aseg, in0=seg_sum, scalar1=inv)

                # x = aseg @ W.T + b  (via PE transpose then matmul with W^T)
                ps_t = ps_t_p.tile([D, S], f32)
                nc.tensor.transpose(ps_t, aseg, ident_v)
                asegT = tail.tile([D, S], f32, tag="asegT")
                nc.scalar.copy(out=asegT, in_=ps_t)
                ps_x = ps_x_p.tile([S, E], f32)
                nc.tensor.matmul(ps_x, lhsT=asegT, rhs=wt_v, start=True, stop=True)
                x = tail.tile([S, E], f32, tag="x")
                nc.vector.tensor_add(out=x, in0=ps_x, in1=bvec_v)

                # LayerNorm
                stats = tail.tile([S, 6], f32, tag="stats")
                nc.vector.bn_stats(out=stats, in_=x)
                mv = tail.tile([S, 2], f32, tag="mv")
                nc.vector.bn_aggr(out=mv, in_=stats)
                rstd = tail.tile([S, 1], f32, tag="rstd")
                nc.scalar.activation(
                    out=rstd, in_=mv[:, 1:2],
                    func=mybir.ActivationFunctionType.Sqrt, bias=eps_sb, scale=1.0,
                )
                nc.vector.reciprocal(out=rstd, in_=rstd)
                y = tail.tile([S, E], f32, tag="y")
                nc.vector.tensor_scalar(
                    out=y, in0=x, scalar1=mv[:, 0:1], scalar2=rstd,
                    op0=AL.subtract, op1=AL.mult,
                )
                nc.vector.tensor_mul(out=y, in0=y, in1=gvec_v)
                nc.vector.tensor_add(out=y, in0=y, in1=beta_v)

                # L2 norm + weighted cosine
                yy = tail.tile([S, E], f32, tag="yy")
                nc.vector.tensor_mul(out=yy, in0=y, in1=y)
                ss = tail.tile([S, 1], f32, tag="ss")
                nc.vector.reduce_sum(out=ss, in_=yy, axis=mybir.AxisListType.X)
                rn = tail.tile([S, 1], f32, tag="rn")
                nc.scalar.activation(out=rn, in_=ss, func=mybir.ActivationFunctionType.Sqrt)
                nc.vector.tensor_scalar_max(out=rn, in0=rn, scalar1=NORM_EPS)
                nc.vector.reciprocal(out=rn, in_=rn)
                pv = tail.tile([S, E], f32, tag="pv")
                vis_v = meta_sb[0:S, MC_VIS + b * E:MC_VIS + (b + 1) * E]
                nc.vector.tensor_mul(out=pv, in0=y, in1=vis_v)
                dot = tail.tile([S, 1], f32, tag="dot")
                nc.vector.reduce_sum(out=dot, in_=pv, axis=mybir.AxisListType.X)
                nc.vector.tensor_mul(out=dot, in0=dot, in1=rn)
                wgt_col = meta_sb[0:S, MC_WGT + b:MC_WGT + b + 1]
                nc.vector.tensor_mul(out=out_sb[:, b:b + 1], in0=dot, in1=wgt_col)

            # stream each batch's accumulation; emit the previous batch's
            # tail AFTER this batch's matmuls so the PE FIFO never blocks on
            # the DVE tail chain (software pipelining, one batch deep)
            ps_tiles = []
            for b in range(BL):
                oh_all = oh_tiles[b]
                ps = ps_seg_p.tile([S, AUGW], f32)
                pslo = ps_lo_p.tile([S, LOW], f32)
                ps_tiles.append((ps, pslo))
                for g in range(NCH // G):
                    aug_sb = aug_p.tile([128, G, AUGW], bf16)
                    nc.sync.dma_start(out=aug_sb, in_=aug[b][:, g * G:(g + 1) * G, :])
                    auglo_sb = aug_p.tile([128, G, LOW], mybir.dt.float8e4, tag="auglo")
                    nc.sync.dma_start(out=auglo_sb, in_=auglo[b][:, g * G:(g + 1) * G, :])
                    for j in range(G):
                        c = g * G + j
                        nc.tensor.matmul(
                            ps, lhsT=oh_all[:, c, :], rhs=aug_sb[:, j, :],
                            start=(c == 0), stop=(c == NCH - 1),
                        )
                        nc.tensor.matmul(
                            pslo, lhsT=oh_all[:, c, :], rhs=auglo_sb[:, j, :],
                            start=(c == 0), stop=(c == NCH - 1),
                        )
                if b >= 1:
                    tail_chain(b - 1, ps_tiles[b - 1])
            tail_chain(BL - 1, ps_tiles[BL - 1])

            nc.sync.dma_start(out=out[:], in_=out_sb)

    if legalize:
        _legalize_multiwait(nc)
    return nc


def _legalize_multiwait(nc):
    """This container's walrus codegen encodes at most ONE sync-wait per TPB
    instruction. Tile emits instructions with several waits; split the
    excess onto standalone wait-only EventSemaphore instructions inserted
    immediately before, on the same engine (sequencers execute per-engine
    in block order, so semantics are identical)."""
    wid = 0
    for fn in nc.m.functions:
        for blk in fn.blocks:
            out_list = []
            for inst in blk.instructions:
                si = inst.sync_info
                if si is not None and len(si.on_wait) > 1:
                    for w in si.on_wait[:-1]:
                        wid += 1
                        nop = mybir.InstEventSemaphore(
                            name=f"W-split-{wid}",
                            engine=inst.engine,
                            ins=[], outs=[],
                            sync_info=mybir.SyncInfo(on_wait=[w], on_update=[]),
                        )
                        out_list.append(nop)
                    inst.sync_info = mybir.SyncInfo(
                        on_wait=[si.on_wait[-1]], on_update=si.on_update)
                out_list.append(inst)
            blk.instructions[:] = out_list


def get_nc():
    global _NC_CACHE
    if _NC_CACHE is None:
        _NC_CACHE = build_nc()
    return _NC_CACHE


def segment_bounds(length):
    """Replicates jnp.linspace(0, L, S+1).astype(int32) as executed by the
    reference on this backend: f32 lerp, then round-half-even f32->s32."""
    lf = np.asarray(length).astype(np.float32)
    i = np.arange(S + 1, dtype=np.float32)
    bf = (i[None, :] * (lf / np.float32(S))[:, None]).astype(np.float32)
    return np.rint(bf).astype(np.int32)


def make_inputs(audio, visual_embed, sync_score, mask, length, seg_mask,
                W, b_lin, gamma, beta):
    audio = np.asarray(audio, dtype=np.float32)
    mask = np.asarray(mask, dtype=np.int32)
    length = np.asarray(length, dtype=np.int32)

    bounds = segment_bounds(length)            # [B, S+1]
    ends = bounds[:, 1:]                       # [B, S]
    t = np.arange(T, dtype=np.int32)
    seg_id = np.empty((B, T), np.int32)
    for bb in range(B):
        s = np.searchsorted(ends[bb], t, side="right").astype(np.int32)
        s[s >= S] = -1
        s[t >= length[bb]] = -1
        s[mask[bb] == 0] = -1
        seg_id[bb] = s
    # [B, 128(p), NCH(c)] with t = c*128 + p
    segid_t = np.ascontiguousarray(
        seg_id.reshape(B, NCH, 128).transpose(0, 2, 1)).astype(np.float32)

    hi = audio.astype(BF16)
    lo = ((audio - hi.astype(np.float32)) * 256.0).astype(ml_dtypes.float8_e4m3)
    # transposed layout: aug[b, p, c, :] = [hi | 1] of frame t = c*128 + p
    # (big contiguous DMA descriptors); lo plane separate in fp8 e4m3 x256
    aug = np.empty((B, 128, NCH, AUGW), dtype=BF16)
    aug[:, :, :, 0:D] = hi.reshape(B, NCH, 128, D).transpose(0, 2, 1, 3)
    aug[:, :, :, D] = BF16(1.0)
    auglo = np.ascontiguousarray(lo.reshape(B, NCH, 128, D).transpose(0, 2, 1, 3))

    visf = np.asarray(visual_embed, dtype=np.float32)
    wgt = np.asarray(sync_score, np.float32) * np.asarray(seg_mask, np.int32).astype(np.float32)

    meta_common = np.zeros((128, META_COLS), np.float32)
    meta_common[:, MC_WT:MC_WT + E] = np.asarray(W, np.float32).T
    meta_common[:, MC_SIOTA:MC_SIOTA + S] = np.arange(S, dtype=np.float32)[None, :]
    meta_common[0:S, MC_BVEC:MC_BVEC + E] = np.asarray(b_lin, np.float32)[None, :]
    meta_common[0:S, MC_GVEC:MC_GVEC + E] = np.asarray(gamma, np.float32)[None, :]
    meta_common[0:S, MC_BETA:MC_BETA + E] = np.asarray(beta, np.float32)[None, :]
    meta_common[0:S, MC_IDENT:MC_IDENT + S] = np.eye(S, dtype=np.float32)

    in_maps = []
    for k in range(NCORES):
        sl = slice(k * BL, (k + 1) * BL)
        meta = meta_common.copy()
        meta[:, MC_SEG:MC_SEG + BL * NCH] = (
            segid_t[sl].transpose(1, 0, 2).reshape(128, BL * NCH))
        meta[0:S, MC_WGT:MC_WGT + BL] = wgt[sl].T
        meta[0:S, MC_VIS:MC_VIS + BL * E] = (
            visf[sl].transpose(1, 0, 2).reshape(S, BL * E))
        in_maps.append({
            "aug": np.ascontiguousarray(aug[sl]),
            "auglo": np.ascontiguousarray(auglo[sl]),
            "meta": meta,
        })
    return in_maps, wgt


def finish(results, wgt):
    total = 0.0
    for r in results:
        total += float(np.asarray(r["out"], dtype=np.float64).sum())
    wsum = max(float(wgt.astype(np.float64).sum()), 1e-6)
    return np.float32(-(total / wsum))


def run(trace=False, **inputs):
    in_maps, wgt = make_inputs(**inputs)
    res = run_bass_kernel_spmd(get_nc(), in_maps, list(range(NCORES)), trace=trace)
    loss = finish(res.results, wgt)
    return np.asarray(loss, dtype=np.float32), res


def kernel(**inputs):
    loss, _ = run(trace=False, **inputs)
    return loss


# revision 28
# speedup vs baseline: 1.0411x; 1.0320x over previous
"""AudioVisualAlignmentLoss kernel for Trainium2 (8 NeuronCores, SPMD).

Strategy: pure data parallelism — batch B=32 is split 4-per-core across 8
cores. Host-side prep is limited to index/layout work:
  * segment bounds replicated bit-exactly from the reference's
    jnp.linspace(...).astype(int32) (round-to-nearest-even on this backend),
  * per-frame segment id with mask/length folded in (-1 = inactive),
  * audio split into a bf16 `hi` plane laid out as [hi | 1] rows plus an
    fp8-e4m3 `lo` residual plane prescaled by 256 (hi + lo/256 ~ fp32 to
    ~2^-12 per element; end-to-end loss error ~3e-6), both in a
    partition-major layout so every DMA descriptor reads 2-4KB contiguous.

Device per chunk of 128 frames: one bf16 one-hot membership tile
[128t, 64s] (built per batch by a single broadcast is_equal) is the shared
matmul stationary; two matmuls accumulate onehot.T @ [hi|1] -> PSUM
[64s, 129] and onehot.T @ lo -> PSUM [64s, 128] (segment sums + counts).
Per batch tail (software-pipelined one batch behind the stream): segment
means, PE transpose, projection with W^T, LayerNorm (bn_stats),
L2-normalize, dot with the visual embedding, sync/seg-mask weighting ->
64 partials. Host sums the 8x[64, 4] partials and divides by the weight sum.

All small tensors ride in one packed "meta" [128, META_COLS] f32 DMA that
is first on the HWDGE ring, so dependent instructions need one DMA
semaphore. _legalize_multiwait splits Tile-emitted multi-wait sync_infos
into standalone EventSemaphore waits (this walrus encodes at most one
sync-wait per TPB instruction).
"""

import numpy as np
import ml_dtypes

import concourse.bass as bass
import concourse.mybir as mybir
import concourse.tile as tile
from concourse.bass_utils import run_bass_kernel_spmd

BF16 = ml_dtypes.bfloat16

B, T, D, S, E = 32, 8192, 128, 64, 128
NCORES = 8
BL = B // NCORES          # batches per core
NCH = T // 128            # 128-frame chunks per batch
G = 16                    # chunks per DMA group
AUGW = D                  # hi plane (bf16), 256B rows
LOW = D + 1                # lo | ones plane (fp8 e4m3, lo prescaled by 256)
LO_SCALE = 1.0 / 256.0
LN_EPS = 1e-5
NORM_EPS = 1e-12

# meta column layout (f32, [128, META_COLS])
MC_WT = 0                  # [128, 128] W^T (wt[d, e] = W[e, d])
MC_SEG = MC_WT + E         # [128, BL*NCH] seg ids, col = b*NCH + c
MC_SIOTA = MC_SEG + BL * NCH   # [128, S] column index
MC_BVEC = MC_SIOTA + S     # [64, E] b_lin broadcast (partitions 0:S)
MC_GVEC = MC_BVEC + E      # [64, E] gamma broadcast
MC_BETA = MC_GVEC + E      # [64, E] beta broadcast
MC_IDENT = MC_BETA + E     # [64, 64] identity
MC_WGT = MC_IDENT + S      # [64, BL] sync*seg_mask, col b
MC_VIS = MC_WGT + BL       # [64, BL*E] visual embed, col = b*E + e
META_COLS = MC_VIS + BL * E

_NC_CACHE = None


def build_nc(legalize=True):
    f32 = mybir.dt.float32
    bf16 = mybir.dt.bfloat16
    AL = mybir.AluOpType

    nc = bass.Bass()
    aug = nc.declare_dram_parameter("aug", [BL, 128, NCH, AUGW], bf16, isOutput=False)
    auglo = nc.declare_dram_parameter("auglo", [BL, 128, NCH, LOW], mybir.dt.float8e4, isOutput=False)
    meta = nc.declare_dram_parameter("meta", [128, META_COLS], f32, isOutput=False)
    out = nc.declare_dram_parameter("out", [S, BL], f32, isOutput=True)

    with tile.TileContext(nc) as tc:
        with (
            tc.tile_pool(name="consts", bufs=1) as consts,
            tc.tile_pool(name="aug_p", bufs=6) as aug_p,
            tc.tile_pool(name="oh_p", bufs=4) as oh_p,
            tc.tile_pool(name="tail", bufs=2) as tail,
            tc.tile_pool(name="ps_seg", bufs=2, space="PSUM") as ps_seg_p,
            tc.tile_pool(name="ps_lo", bufs=2, space="PSUM") as ps_lo_p,
            tc.tile_pool(name="ps_t", bufs=2, space="PSUM") as ps_t_p,
            tc.tile_pool(name="ps_x", bufs=2, space="PSUM") as ps_x_p,
        ):
            meta_sb = consts.tile([128, META_COLS], f32)
            nc.sync.dma_start(out=meta_sb, in_=meta[:])
            wt_v = meta_sb[:, MC_WT:MC_WT + E]
            siota_v = meta_sb[:, MC_SIOTA:MC_SIOTA + S]
            bvec_v = meta_sb[0:S, MC_BVEC:MC_BVEC + E]
            gvec_v = meta_sb[0:S, MC_GVEC:MC_GVEC + E]
            beta_v = meta_sb[0:S, MC_BETA:MC_BETA + E]
            ident_v = meta_sb[0:S, MC_IDENT:MC_IDENT + S]

            eps_sb = consts.tile([S, 1], f32)
            nc.vector.memset(eps_sb, LN_EPS)
            out_sb = consts.tile([S, BL], f32)

            # one-hot membership, all batches up front, in 16-chunk pieces:
            # oh_all[p, c, s] = (segid[p, c] == s)
            oh_tiles = []
            for b in range(BL):
                oh_all = oh_p.tile([128, NCH, S], bf16)
                oh_tiles.append(oh_all)
                pieces = [4, 12, 16, 16, 16] if b == 0 else [16, 16, 16, 16]
                c0 = 0
                for H in pieces:
                    c1 = c0 + H
                    seg_h = meta_sb[:, MC_SEG + b * NCH + c0:MC_SEG + b * NCH + c1]
                    nc.vector.tensor_tensor(
                        out=oh_all[:, c0:c1, :],
                        in0=seg_h.rearrange("p (c o) -> p c o", o=1).broadcast_to([128, H, S]),
                        in1=siota_v.rearrange("p (o s) -> p o s", o=1).broadcast_to([128, H, S]),
                        op=AL.is_equal,
                    )
                    c0 = c1

            def tail_chain(b, ps_pair):
                ps, pslo = ps_pair
                # segment means: (hi-part + lo-part/256) / max(count, 1)
                # (PSUM has one DVE read port: bounce hi to SBUF via ACT first)
                seg_hi = tail.tile([S, D], f32, tag="seg_hi")
                nc.scalar.copy(out=seg_hi, in_=ps[:, 0:D])
                lo_s = tail.tile([S, D], f32, tag="lo_s")
                nc.vector.tensor_scalar_mul(out=lo_s, in0=pslo[:, 0:D], scalar1=LO_SCALE)
                seg_sum = tail.tile([S, D], f32, tag="seg_sum")
                nc.vector.tensor_add(out=seg_sum, in0=seg_hi, in1=lo_s)
                inv = tail.tile([S, 1], f32, tag="inv")
                nc.vector.tensor_scalar_max(out=inv, in0=pslo[:, D:D + 1], scalar1=1.0)
                nc.vector.reciprocal(out=inv, in_=inv)
                aseg = tail.tile([S, D], f32, tag="aseg")
                nc.vector.tensor_scalar_mul(out=aseg, in0=seg_sum, scalar1=inv)

                # x = aseg @ W.T + b  (via PE transpose then matmul with W^T)
                ps_t = ps_t_p.tile([D, S], f32)
                nc.tensor.transpose(ps_t, aseg, ident_v)
                asegT = tail.tile([D, S], f32, tag="asegT")
                nc.scalar.copy(out=asegT, in_=ps_t)
                ps_x = ps_x_p.tile([S, E], f32)
                nc.tensor.matmul(ps_x, lhsT=asegT, rhs=wt_v, start=True, stop=True)
                x = tail.tile([S, E], f32, tag="x")
                nc.vector.tensor_add(out=x, in0=ps_x, in1=bvec_v)

                # LayerNorm
                stats = tail.tile([S, 6], f32, tag="stats")
                nc.vector.bn_stats(out=stats, in_=x)
                mv = tail.tile([S, 2], f32, tag="mv")
                nc.vector.bn_aggr(out=mv, in_=stats)
                rstd = tail.tile([S, 1], f32, tag="rstd")
                nc.scalar.activation(
                    out=rstd, in_=mv[:, 1:2],
                    func=mybir.ActivationFunctionType.Sqrt, bias=eps_sb, scale=1.0,
                )
                nc.vector.reciprocal(out=rstd, in_=rstd)
                y = tail.tile([S, E], f32, tag="y")
                nc.vector.tensor_scalar(
                    out=y, in0=x, scalar1=mv[:, 0:1], scalar2=rstd,
                    op0=AL.subtract, op1=AL.mult,
                )
                nc.vector.tensor_mul(out=y, in0=y, in1=gvec_v)
                nc.vector.tensor_add(out=y, in0=y, in1=beta_v)

                # L2 norm + weighted cosine
                yy = tail.tile([S, E], f32, tag="yy")
                nc.vector.tensor_mul(out=yy, in0=y, in1=y)
                ss = tail.tile([S, 1], f32, tag="ss")
                nc.vector.reduce_sum(out=ss, in_=yy, axis=mybir.AxisListType.X)
                rn = tail.tile([S, 1], f32, tag="rn")
                nc.scalar.activation(out=rn, in_=ss, func=mybir.ActivationFunctionType.Sqrt)
                nc.vector.tensor_scalar_max(out=rn, in0=rn, scalar1=NORM_EPS)
                nc.vector.reciprocal(out=rn, in_=rn)
                pv = tail.tile([S, E], f32, tag="pv")
                vis_v = meta_sb[0:S, MC_VIS + b * E:MC_VIS + (b + 1) * E]
                nc.vector.tensor_mul(out=pv, in0=y, in1=vis_v)
                dot = tail.tile([S, 1], f32, tag="dot")
                nc.vector.reduce_sum(out=dot, in_=pv, axis=mybir.AxisListType.X)
                nc.vector.tensor_mul(out=dot, in0=dot, in1=rn)
                wgt_col = meta_sb[0:S, MC_WGT + b:MC_WGT + b + 1]
                nc.vector.tensor_mul(out=out_sb[:, b:b + 1], in0=dot, in1=wgt_col)

            # stream each batch's accumulation; emit the previous batch's
            # tail AFTER this batch's matmuls so the PE FIFO never blocks on
            # the DVE tail chain (software pipelining, one batch deep)
            ps_tiles = []
            for b in range(BL):
                oh_all = oh_tiles[b]
                ps = ps_seg_p.tile([S, AUGW], f32)
                pslo = ps_lo_p.tile([S, LOW], f32)
                ps_tiles.append((ps, pslo))
                for g in range(NCH // G):
                    aug_sb = aug_p.tile([128, G, AUGW], bf16)
                    nc.sync.dma_start(out=aug_sb, in_=aug[b][:, g * G:(g + 1) * G, :])
                    auglo_sb = aug_p.tile([128, G, LOW], mybir.dt.float8e4, tag="auglo")
                    nc.sync.dma_start(out=auglo_sb, in_=auglo[b][:, g * G:(g + 1) * G, :])
                    for j in range(G):
                        c = g * G + j
                        nc.tensor.matmul(
                            ps, lhsT=oh_all[:, c, :], rhs=aug_sb[:, j, :],
                            start=(c == 0), stop=(c == NCH - 1),
                        )
                        nc.tensor.matmul(
                            pslo, lhsT=oh_all[:, c, :], rhs=auglo_sb[:, j, :],
                            start=(c == 0), stop=(c == NCH - 1),
                        )
                if b >= 1:
                    tail_chain(b - 1, ps_tiles[b - 1])
            tail_chain(BL - 1, ps_tiles[BL - 1])

            nc.sync.dma_start(out=out[:], in_=out_sb)

    if legalize:
        _legalize_multiwait(nc)
    return nc


def _legalize_multiwait(nc):
    """This container's walrus codegen encodes at most ONE sync-wait per TPB
    instruction. Tile emits instructions with several waits; split the
    excess onto standalone wait-only EventSemaphore instructions inserted
    immediately before, on the same engine (sequencers execute per-engine
    in block order, so semantics are identical)."""
    wid = 0
    for fn in nc.m.functions:
        for blk in fn.blocks:
            out_list = []
            for inst in blk.instructions:
                si = inst.sync_info
                if si is not None and len(si.on_wait) > 1:
                    for w in si.on_wait[:-1]:
                        wid += 1
                        nop = mybir.InstEventSemaphore(
                            name=f"W-split-{wid}",
                            engine=inst.engine,
                            ins=[], outs=[],
                            sync_info=mybir.SyncInfo(on_wait=[w], on_update=[]),
                        )
                        out_list.append(nop)
                    inst.sync_info = mybir.SyncInfo(
                        on_wait=[si.on_wait[-1]], on_update=si.on_update)
                out_list.append(inst)
            blk.instructions[:] = out_list


def get_nc():
    global _NC_CACHE
    if _NC_CACHE is None:
        _NC_CACHE = build_nc()
    return _NC_CACHE


def segment_bounds(length):
    """Replicates jnp.linspace(0, L, S+1).astype(int32) as executed by the
    reference on this backend: f32 lerp, then round-half-even f32->s32."""
    lf = np.asarray(length).astype(np.float32)
    i = np.arange(S + 1, dtype=np.float32)
    bf = (i[None, :] * (lf / np.float32(S))[:, None]).astype(np.float32)
    return np.rint(bf).astype(np.int32)


def make_inputs(audio, visual_embed, sync_score, mask, length, seg_mask,
                W, b_lin, gamma, beta):
    audio = np.asarray(audio, dtype=np.float32)
    mask = np.asarray(mask, dtype=np.int32)
    length = np.asarray(length, dtype=np.int32)

    bounds = segment_bounds(length)            # [B, S+1]
    ends = bounds[:, 1:]                       # [B, S]
    t = np.arange(T, dtype=np.int32)
    seg_id = np.empty((B, T), np.int32)
    for bb in range(B):
        s = np.searchsorted(ends[bb], t, side="right").astype(np.int32)
        s[s >= S] = -1
        s[t >= length[bb]] = -1
        s[mask[bb] == 0] = -1
        seg_id[bb] = s
    # [B, 128(p), NCH(c)] with t = c*128 + p
    segid_t = np.ascontiguousarray(
        seg_id.reshape(B, NCH, 128).transpose(0, 2, 1)).astype(np.float32)

    hi = audio.astype(BF16)
    lo = ((audio - hi.astype(np.float32)) * 256.0).astype(ml_dtypes.float8_e4m3)
    # transposed layout: aug[b, p, c, :] = [hi | 1] of frame t = c*128 + p
    # (big contiguous DMA descriptors); lo plane separate in fp8 e4m3 x256
    aug = np.ascontiguousarray(
        hi.reshape(B, NCH, 128, D).transpose(0, 2, 1, 3))
    auglo = np.empty((B, 128, NCH, LOW), dtype=ml_dtypes.float8_e4m3)
    auglo[:, :, :, 0:D] = lo.reshape(B, NCH, 128, D).transpose(0, 2, 1, 3)
    auglo[:, :, :, D] = 1.0

    visf = np.asarray(visual_embed, dtype=np.float32)
    wgt = np.asarray(sync_score, np.float32) * np.asarray(seg_mask, np.int32).astype(np.float32)

    meta_common = np.zeros((128, META_COLS), np.float32)
    meta_common[:, MC_WT:MC_WT + E] = np.asarray(W, np.float32).T
    meta_common[:, MC_SIOTA:MC_SIOTA + S] = np.arange(S, dtype=np.float32)[None, :]
    meta_common[0:S, MC_BVEC:MC_BVEC + E] = np.asarray(b_lin, np.float32)[None, :]
    meta_common[0:S, MC_GVEC:MC_GVEC + E] = np.asarray(gamma, np.float32)[None, :]
    meta_common[0:S, MC_BETA:MC_BETA + E] = np.asarray(beta, np.float32)[None, :]
    meta_common[0:S, MC_IDENT:MC_IDENT + S] = np.eye(S, dtype=np.float32)

    in_maps = []
    for k in range(NCORES):
        sl = slice(k * BL, (k + 1) * BL)
        meta = meta_common.copy()
        meta[:, MC_SEG:MC_SEG + BL * NCH] = (
            segid_t[sl].transpose(1, 0, 2).reshape(128, BL * NCH))
        meta[0:S, MC_WGT:MC_WGT + BL] = wgt[sl].T
        meta[0:S, MC_VIS:MC_VIS + BL * E] = (
            visf[sl].transpose(1, 0, 2).reshape(S, BL * E))
        in_maps.append({
            "aug": np.ascontiguousarray(aug[sl]),
            "auglo": np.ascontiguousarray(auglo[sl]),
            "meta": meta,
        })
    return in_maps, wgt


def finish(results, wgt):
    total = 0.0
    for r in results:
        total += float(np.asarray(r["out"], dtype=np.float64).sum())
    wsum = max(float(wgt.astype(np.float64).sum()), 1e-6)
    return np.float32(-(total / wsum))


def run(trace=False, **inputs):
    in_maps, wgt = make_inputs(**inputs)
    res = run_bass_kernel_spmd(get_nc(), in_maps, list(range(NCORES)), trace=trace)
    loss = finish(res.results, wgt)
    return np.asarray(loss, dtype=np.float32), res


def kernel(**inputs):
    loss, _ = run(trace=False, **inputs)
    return loss
